# revision 57
# baseline (speedup 1.0000x reference)
"""AttentionBlock (GroupNorm + MHA + proj + residual) on 8 trn2 NeuronCores.

Sharding: core = (batch b, L-half lh).  Each core gets the full x[b] (rolled so
its local query half is always columns 0..1024 -- softmax/groupnorm are
permutation invariant over L, so all 8 cores run the *same* graph with zero
collectives), computes full GroupNorm + full K/V, Q only for its local half,
attention for all 8 heads over its 1024 query positions, proj + residual, and
writes a (512, 1024) output shard.

Matmul operands are bf16 (PE streams 1 col/cycle; fp32 PSUM accumulation).
The softmax exp is split across engines: ScalarE runs native EXP on most
key-tiles, the DVE computes the rest with a Schraudolph fast-exp (one
tensor_scalar producing the bf16 bit pattern as int16).  The K/Q/vT builds,
softmax divides and proj chunks are interleaved into the attention kt loops
so the PE never drains (HAM stays at full clock) and the exp engines start
as early as possible.
"""

import sys

for _p in ("/opt/trn_rl_repo", "/root/.axon_site/_ro/trn_rl_repo"):
    if _p not in sys.path:
        sys.path.insert(0, _p)

import math
import os

import numpy as np
import ml_dtypes

import concourse.bass as bass
import concourse.bacc as bacc
import concourse.tile as tile
from concourse import mybir

C = 512          # channels
L = 2048         # sequence length
LH = 1024        # local query half
B = 4            # batch
H = 8            # heads
D = 64           # head dim
G = 8            # groups
EPS = 1e-5
NT = C // 128    # channel tiles (4)
NKT = L // 128   # key-position tiles (16)
SCALE = D ** -0.5

# Schraudolph fast-exp in fp8e4: p = bitcast_e4m3(u8(s*A8 + B8)).
# B8 shifts all weights by 2^((B8-56)/8) -- softmax-invariant as long as
# the ScalarE native-exp tiles apply the matching log-space bias.
SCH_A8 = 8.0 * math.log2(math.e) * SCALE
# B8=38 keeps the top score (~6.6 scaled) under the HW e4m3 max of 240
# (IEEE-style e4m3 with inf, not e4m3fn) on both exp paths
SCH_B8 = 38.0
EXP_BIAS = (SCH_B8 - 56.0) / 8.0 * math.log(2.0)
# exp engine split: ScalarE takes j=0 halves, DVE j=1, except S_BOTH
# key-tiles where ScalarE takes both halves (D_BOTH the reverse).
S_BOTH = frozenset((6, 12))
D_BOTH = frozenset(())
VE = 80  # padded per-head stride in the DoubleRow vT layout (16B-aligned)

f32 = mybir.dt.float32
bf16 = mybir.dt.bfloat16
fp8 = mybir.dt.float8e4
u8 = mybir.dt.uint8
AF = mybir.ActivationFunctionType
ALU = mybir.AluOpType
DR = mybir.MatmulPerfMode.DoubleRow


def build_graph():
    nc = bacc.Bacc(None, target_bir_lowering=False)

    x_e = nc.declare_dram_parameter("x", [C, L], bf16, isOutput=False)
    xr_e = nc.declare_dram_parameter("xr", [C, LH], f32, isOutput=False)
    w_e = nc.declare_dram_parameter("wqkvt", [C, 3 * C], bf16, isOutput=False)
    pw_e = nc.declare_dram_parameter("pwt", [C, C], bf16, isOutput=False)
    # packed per-channel vectors: [128, NT, 5] = (nw, nb, qb, kb, pbe)
    vcs_e = nc.declare_dram_parameter("vcs", [128, NT, 5], f32, isOutput=False)
    indt_e = nc.declare_dram_parameter("indt", [G, NT * 128], bf16, isOutput=False)
    out_e = nc.declare_dram_parameter("out", [C, LH], f32, isOutput=True)

    with tile.TileContext(nc) as tc:
        with (
            tc.tile_pool(name="cst", bufs=1) as cst,
            tc.tile_pool(name="big", bufs=1) as big,
            tc.tile_pool(name="sm", bufs=2) as sm,
            tc.tile_pool(name="wp", bufs=1) as wp,
            tc.tile_pool(name="hp", bufs=1) as hp,
            tc.tile_pool(name="pp", bufs=6) as pp,
            tc.tile_pool(name="op", bufs=3) as op,
            tc.tile_pool(name="un", bufs=4) as unp,
            tc.tile_pool(name="xrp", bufs=1) as xrp,
            tc.tile_pool(name="psT", bufs=2, space="PSUM") as psT,
            tc.tile_pool(name="psV", bufs=2, space="PSUM") as psV,
            tc.tile_pool(name="psQ", bufs=2, space="PSUM") as psQ,
        ):
            # ---- persistent SBUF tensors -------------------------------
            x_t = [big.tile([128, L], bf16, name=f"x{t}", tag=f"x{t}")
                   for t in range(NT)]
            k_t = [big.tile([128, L], bf16, name=f"k{t}", tag=f"k{t}")
                   for t in range(NT)]
            q_t = [big.tile([128, LH], bf16, name=f"q{t}", tag=f"q{t}")
                   for t in range(NT)]
            # vT2: per key-tile PAIR, fp8 DoubleRow layout (l_part, 2, H*VE):
            # head block = 64 v dims + ones col + pad to a 16B-aligned stride
            vT2 = [big.tile([128, 2, H * VE], fp8, name=f"v{t}", tag=f"v{t}")
                   for t in range(NKT // 2)]
            attn_t = [big.tile([128, LH], bf16, name=f"a{t}", tag=f"a{t}")
                      for t in range(NT)]
            pw_t = [big.tile([128, C], bf16, name=f"pw{t}", tag=f"pw{t}")
                    for t in range(NT)]
            w_t = [wp.tile([128, 3 * C], bf16, name=f"w{t}", tag=f"w{t}")
                   for t in range(NT)]
            vcs = cst.tile([128, NT, 5], f32, name="vcs", tag="vcs")
            indT_all = cst.tile([G, NT * 128], bf16, name="indT_all", tag="indTa")
            xr_t = [xrp.tile([128, LH], f32, name=f"xr{i}", tag=f"xr{i}")
                    for i in range(NT)]

            # x rides two DMA queues in parallel: tiles 0/1 on sync (DVE
            # stats), tiles 2/3 on the idle tensor queue (ScalarE / DVE
            # stats).  Constants + weights follow on sync; pw / xr follow
            # on the tensor queue (needed late).
            for t in range(2):
                for s in range(2):
                    nc.sync.dma_start(
                        out=x_t[t][:, s * LH:(s + 1) * LH],
                        in_=x_e[t * 128:(t + 1) * 128, s * LH:(s + 1) * LH],
                    )
            for t in range(2, NT):
                for s in range(2):
                    nc.scalar.dma_start(
                        out=x_t[t][:, s * LH:(s + 1) * LH],
                        in_=x_e[t * 128:(t + 1) * 128, s * LH:(s + 1) * LH],
                    )
            nc.sync.dma_start(out=vcs, in_=vcs_e[:, :, :])
            nc.sync.dma_start(out=indT_all, in_=indt_e[:, :])
            for t in range(NT):
                nc.sync.dma_start(out=w_t[t], in_=w_e[t * 128:(t + 1) * 128, :])
            for t in range(NT):
                nc.gpsimd.dma_start(
                    out=pw_t[t], in_=pw_e[t * 128:(t + 1) * 128, :])
            for mo in range(NT):
                nc.gpsimd.dma_start(
                    out=xr_t[mo], in_=xr_e[mo * 128:(mo + 1) * 128, :])

            nw_t = [vcs[:, t, 0:1] for t in range(NT)]
            nb_t = [vcs[:, t, 1:2] for t in range(NT)]
            qb_t = [vcs[:, t, 2:3] for t in range(NT)]
            kb_t = [vcs[:, t, 3:4] for t in range(NT)]
            pbe_t = [vcs[:, t, 4:5] for t in range(NT)]
            indT = [indT_all[:, t * 128:(t + 1) * 128] for t in range(NT)]

            eps_t = cst.tile([G, 1], f32, name="eps", tag="eps")
            nc.vector.memset(eps_t, EPS)
            ebias_t = cst.tile([128, 1], f32, name="ebias", tag="ebias")
            nc.vector.memset(ebias_t, EXP_BIAS)
            ones64 = cst.tile([1, D], bf16, name="ones64", tag="ones64")
            nc.vector.memset(ones64, 1.0)

            # group indicator matrices for cross-partition stats
            ind = [cst.tile([128, G], bf16, name=f"ind{t}", tag=f"ind{t}")
                   for t in range(NT)]
            for t in range(NT):
                nc.vector.memset(ind[t], 0.0)
                nc.vector.memset(ind[t][0:64, 2 * t:2 * t + 1], 1.0 / D)
                nc.vector.memset(ind[t][64:128, 2 * t + 1:2 * t + 2], 1.0 / D)

            # ---- phase A: groupnorm stats ------------------------------
            # s2[t] = (E[x], E[x^2]) per channel, bf16.  Tiles 0/1/3 via
            # DVE bn_stats, tile 2 via ScalarE activation accumulators --
            # the two engines run in parallel behind the two DMA queues.
            stats2 = [None] * NT
            t = 2
            sx2 = sm.tile([128, 2], f32, name="sx2", tag="sx2")
            sq2 = sm.tile([128, 2], f32, name="sq2", tag="sq2")
            for half in range(2):
                xs = x_t[t][:, half * LH:(half + 1) * LH]
                scr = sm.tile([128, LH], bf16, name=f"scr{half}", tag="scr")
                nc.scalar.activation(out=scr, in_=xs, func=AF.Square,
                                     accum_out=sq2[:, half:half + 1])
                nc.scalar.activation(out=scr, in_=xs, func=AF.Identity,
                                     accum_out=sx2[:, half:half + 1])
            s2_2 = sm.tile([128, 2], bf16, name="s2_2", tag="s2_2")
            sxt = sm.tile([128, 2], f32, name="sxt", tag="sxt")
            nc.vector.tensor_add(sxt[:, 0:1], sx2[:, 0:1], sx2[:, 1:2])
            nc.vector.tensor_add(sxt[:, 1:2], sq2[:, 0:1], sq2[:, 1:2])
            nc.vector.tensor_scalar(
                out=s2_2, in0=sxt, scalar1=1.0 / L, scalar2=None, op0=ALU.mult)
            stats2[2] = s2_2
            for t in (0, 1, 3):
                bn = sm.tile([128, L // 512, 6], f32, name="bn", tag="bn")
                for s in range(L // 512):
                    nc.vector.bn_stats(
                        out=bn[:, s, :], in_=x_t[t][:, s * 512:(s + 1) * 512]
                    )
                mv = sm.tile([128, 2], f32, name=f"mv{t}", tag=f"mv{t}")
                nc.vector.bn_aggr(out=mv, in_=bn)
                s2 = sm.tile([128, 2], bf16, name=f"s2{t}", tag=f"s2{t}")
                nc.vector.tensor_copy(out=s2[:, 0:1], in_=mv[:, 0:1])
                nc.vector.tensor_mul(s2[:, 1:2], mv[:, 0:1], mv[:, 0:1])
                nc.vector.tensor_add(s2[:, 1:2], s2[:, 1:2], mv[:, 1:2])
                stats2[t] = s2

            # PE warmup: junk matmuls during the stats wait flip the HAM
            # clock gate to 8/8 before the real K/Q/vT stream starts
            jp = psQ.tile([G, 512], f32, name="jp", tag="ps")
            for wu in range(40):
                nc.tensor.matmul(
                    jp, ind[wu % 2], x_t[0][:, 0:512], start=True, stop=True)

            gps = psQ.tile([G, 2], f32, name="gps", tag="ps")
            for t in range(NT):
                nc.tensor.matmul(
                    gps, ind[t], stats2[t], start=(t == 0), stop=(t == NT - 1)
                )
            mean_g = sm.tile([G, 1], f32, name="mean_g", tag="mean_g")
            nc.vector.tensor_copy(out=mean_g, in_=gps[:, 0:1])
            var_g = sm.tile([G, 1], f32, name="var_g", tag="var_g")
            nc.vector.tensor_mul(var_g, mean_g, mean_g)
            nc.vector.tensor_sub(var_g, gps[:, 1:2], var_g)
            gsb = sm.tile([G, 2], bf16, name="gsb", tag="gsb")
            nc.vector.tensor_copy(out=gsb[:, 0:1], in_=mean_g)
            std_g = sm.tile([G, 1], f32, name="std_g", tag="std_g")
            nc.scalar.activation(
                out=std_g, in_=var_g, func=AF.Sqrt, bias=eps_t, scale=1.0
            )
            with nc.allow_low_precision(reason="groupnorm rstd in bf16; 0.4% scale ok"):
                nc.vector.reciprocal(out=gsb[:, 1:2], in_=std_g)

            h_t = [hp.tile([128, L], bf16, name=f"h{t}", tag=f"h{t}")
                   for t in range(NT)]
            A_t, B_t = [], []
            for t in range(NT):
                bc = psQ.tile([128, 2], f32, name="bc", tag="ps")
                nc.tensor.matmul(bc, indT[t], gsb, start=True, stop=True)
                A = sm.tile([128, 1], f32, name=f"A{t}", tag=f"A{t}")
                Bt = sm.tile([128, 1], f32, name=f"Bt{t}", tag=f"Bt{t}")
                nc.vector.tensor_mul(A, nw_t[t], bc[:, 1:2])
                nc.vector.tensor_mul(Bt, bc[:, 0:1], A)
                nc.vector.tensor_sub(Bt, nb_t[t], Bt)
                A_t.append(A)
                B_t.append(Bt)
            # h = A*x + B, slab-major, alternating ScalarE / DVE
            for s in range(L // 512):
                for t in range(NT):
                    h_slab = h_t[t][:, s * 512:(s + 1) * 512]
                    x_slab = x_t[t][:, s * 512:(s + 1) * 512]
                    if (s + t) % 2 == 0:
                        nc.scalar.activation(
                            out=h_slab, in_=x_slab, func=AF.Identity,
                            bias=B_t[t], scale=A_t[t],
                        )
                    else:
                        nc.vector.tensor_scalar(
                            out=h_slab, in0=x_slab, scalar1=A_t[t],
                            scalar2=B_t[t], op0=ALU.mult, op1=ALU.add,
                        )

            # ---- QKV build helpers -------------------------------------
            def emit_k_part(mt, nk):
                ps = psQ.tile([128, 512], f32, name=f"psk{mt}{nk}", tag="ps")
                for ct in range(NT):
                    nc.tensor.matmul(
                        ps,
                        w_t[ct][:, C + mt * 128:C + (mt + 1) * 128],
                        h_t[ct][:, nk * 512:(nk + 1) * 512],
                        start=(ct == 0), stop=(ct == NT - 1),
                    )
                dst = k_t[mt][:, nk * 512:(nk + 1) * 512]
                if nk % 2 == 0:
                    nc.scalar.add(out=dst, in_=ps, add=kb_t[mt])
                else:
                    nc.vector.tensor_scalar(
                        out=dst, in0=ps, scalar1=kb_t[mt], scalar2=None,
                        op0=ALU.add,
                    )

            def emit_q_part(mt, nq):
                ps = psQ.tile([128, 512], f32, name=f"psq{mt}{nq}", tag="ps")
                for ct in range(NT):
                    nc.tensor.matmul(
                        ps,
                        w_t[ct][:, mt * 128:(mt + 1) * 128],
                        h_t[ct][:, nq * 512:(nq + 1) * 512],
                        start=(ct == 0), stop=(ct == NT - 1),
                    )
                dst = q_t[mt][:, nq * 512:(nq + 1) * 512]
                if nq % 2 == 0:
                    nc.scalar.add(out=dst, in_=ps, add=qb_t[mt])
                else:
                    nc.vector.tensor_scalar(
                        out=dst, in0=ps, scalar1=qb_t[mt], scalar2=None,
                        op0=ALU.add,
                    )

            def emit_vt(lt):
                ps = psQ.tile([128, 512], f32, name=f"psv{lt}", tag="ps")
                for ct in range(NT):
                    nc.tensor.matmul(
                        ps,
                        h_t[ct][:, lt * 128:(lt + 1) * 128],
                        w_t[ct][:, 2 * C:3 * C],
                        start=(ct == 0), stop=(ct == NT - 1),
                    )
                dst = vT2[lt // 2][:, lt % 2, :].rearrange(
                    "p (h e) -> p h e", e=VE)
                src = ps.rearrange("p (h e) -> p h e", e=D)
                if lt % 2 == 0:
                    nc.scalar.copy(out=dst[:, :, 0:D], in_=src)
                else:
                    nc.vector.tensor_copy(out=dst[:, :, 0:D], in_=src)
                nc.vector.memset(dst[:, :, D:D + 1], 1.0)

            # K/Q for head pair 0, first few vT tiles; the rest interleaves
            # into the attention stream below (fills PE exp-wait gaps).
            emit_k_part(0, 0)
            emit_k_part(0, 1)
            emit_q_part(0, 0)
            emit_vt(0)
            emit_vt(1)

            # ---- phase B: attention + proj -----------------------------
            def release_av(lc, pr, av):
                # free both PSUM accumulators; rec = 1/denominator (fp32)
                uns, recs = [], []
                for j in range(2):
                    un = unp.tile([D, 512], bf16, name=f"un{lc}{pr}{j}", tag="un")
                    rec = sm.tile([1, 512], f32, name=f"rec{lc}{pr}{j}",
                                  tag="rec", bufs=4)
                    # custom-DVE ops read garbage from PSUM on HW -> stage
                    # the denominator row through SBUF first
                    den = sm.tile([1, 512], f32, name=f"den{lc}{pr}{j}",
                                  tag="den", bufs=4)
                    if j == 0:
                        nc.scalar.copy(out=den, in_=av[j][D:D + 1, :])
                        nc.scalar.copy(out=un, in_=av[j][0:D, :])
                    else:
                        nc.vector.tensor_copy(out=den, in_=av[j][D:D + 1, :])
                        nc.vector.tensor_copy(out=un, in_=av[j][0:D, :])
                    nc.vector.reciprocal_approx_fast(out=rec, in_=den)
                    uns.append(un)
                    recs.append(rec)
                return uns, recs

            junk_n = [0]
            junk_tgt = [None]

            def new_junk_target():
                junk_n[0] += 1
                junk_tgt[0] = psQ.tile(
                    [G, 512], f32, name=f"jk{junk_n[0]}", tag="ps")

            def emit_junk():
                junk_n[0] += 1
                nc.tensor.matmul(
                    junk_tgt[0], ind[junk_n[0] % 2], x_t[0][:, 0:512],
                    start=True, stop=True)

            def emit_divide_j(lc, pr, uns, recs, j, fast=False):
                recb = sm.tile([1, 512], bf16, name=f"recb{lc}{pr}{j}",
                               tag="recb", bufs=4)
                if fast:
                    nc.vector.tensor_copy(out=recb, in_=recs[j])
                else:
                    nc.gpsimd.tensor_copy(out=recb, in_=recs[j])
                rb = psQ.tile([D, 512], f32, name=f"rb{lc}{pr}{j}", tag="ps")
                nc.tensor.matmul(rb, ones64, recb, start=True, stop=True)
                nc.vector.tensor_mul(
                    attn_t[pr][j * 64:j * 64 + 64, lc * 512:(lc + 1) * 512],
                    uns[j], rb,
                )

            def emit_proj_chunk(lc, mo):
                pj = psQ.tile([128, 512], f32, name=f"pj{lc}{mo}", tag="ps")
                for ct in range(NT):
                    nc.tensor.matmul(
                        pj,
                        pw_t[ct][:, mo * 128:(mo + 1) * 128],
                        attn_t[ct][:, lc * 512:(lc + 1) * 512],
                        start=(ct == 0), stop=(ct == NT - 1),
                    )
                o = op.tile([128, 512], f32, name=f"o{lc}{mo}", tag="o")
                nc.vector.scalar_tensor_tensor(
                    out=o, in0=pj, scalar=pbe_t[mo],
                    in1=xr_t[mo][:, lc * 512:(lc + 1) * 512],
                    op0=ALU.add, op1=ALU.add,
                )
                nc.sync.dma_start(
                    out=out_e[mo * 128:(mo + 1) * 128,
                              lc * 512:(lc + 1) * 512],
                    in_=o,
                )

            # per-pair filler schedule: kt -> [callables]; pre-AV junk
            # bridges the PSUM-release stall at pair boundaries so the
            # HAM clock gate never sees an idle window
            def pair_fillers(lc, pr):
                f = {}
                pre = {}

                def add(kt, fn):
                    f.setdefault(kt, []).append(fn)

                def addp(kt, n):
                    pre.setdefault(kt, []).extend([emit_junk] * n)

                if lc == 0:
                    if pr == 0:
                        # remaining K0/Q0/vT builds + K1/Q1, ordered so
                        # every tile lands a few kt before first use
                        sched = {
                            0: [lambda: emit_vt(2), lambda: emit_vt(3)],
                            1: [lambda: emit_vt(4)],
                            2: [lambda: emit_k_part(0, 2), lambda: emit_vt(5)],
                            3: [lambda: emit_vt(6)],
                            4: [lambda: emit_k_part(0, 3), lambda: emit_vt(7)],
                            5: [lambda: emit_vt(8)],
                            6: [lambda: emit_vt(9)],
                            7: [lambda: emit_k_part(1, 0), lambda: emit_vt(10)],
                            8: [lambda: emit_vt(11)],
                            9: [lambda: emit_k_part(1, 1), lambda: emit_vt(12)],
                            10: [lambda: emit_q_part(1, 0), lambda: emit_vt(13)],
                            11: [lambda: emit_k_part(1, 2), lambda: emit_vt(14)],
                            12: [lambda: emit_k_part(1, 3), lambda: emit_vt(15)],
                            13: [lambda: emit_q_part(1, 1)],
                            15: [lambda: emit_q_part(0, 1)],
                        }
                        for kkt, fns in sched.items():
                            for fn in fns:
                                add(kkt, fn)
                    elif pr < NT - 1:
                        for idx, kkt in enumerate((1, 3, 7, 9)):
                            add(kkt, lambda nk=idx, m=pr + 1: emit_k_part(m, nk))
                        add(11, lambda m=pr + 1: emit_q_part(m, 0))
                        add(13, lambda m=pr + 1: emit_q_part(m, 1))
                    else:
                        addp(0, 2)
                        addp(1, 2)
                        for kt in range(2, NKT):
                            addp(kt, 1)
                else:
                    addp(0, 2)
                    addp(1, 2)
                    for kt in range(2, NKT):
                        addp(kt, 1)
                return f, pre

            pending = None       # (lc, pr, uns, recs) of the previous pair
            proj_chunks = []     # deferred proj chunk callables
            for lc in range(LH // 512):
                for pr in range(NT):  # head pair (2pr, 2pr+1)
                    fillers, pre_fillers = pair_fillers(lc, pr)
                    if pre_fillers:
                        new_junk_target()
                    av = [psV.tile([D + 1, 512], f32, name=f"av{lc}{pr}{j}",
                                   tag="av") for j in range(2)]

                    def emit_av(ktp, p2_pair):
                        # fp8 DoubleRow: one matmul covers 2 key-tiles
                        for j in range(2):
                            h2 = 2 * pr + j
                            nc.tensor.matmul(
                                av[j],
                                vT2[ktp][:, :, h2 * VE:h2 * VE + D + 1],
                                p2_pair[j],
                                start=(ktp == 0), stop=(ktp == NKT // 2 - 1),
                                perf_mode=DR,
                            )

                    av_q = []
                    p2_cur = None
                    for kt in range(NKT):
                        if kt % 2 == 0:
                            p2_cur = [
                                pp.tile([128, 2, 512], fp8, tag="p", bufs=8,
                                        name=f"p{lc}{pr}{kt}{j}")
                                for j in range(2)
                            ]
                        for j in range(2):
                            hp0 = j * 64
                            stj = psT.tile([128, 512], f32, tag="st", bufs=4,
                                           name=f"st{lc}{pr}{kt}{j}")
                            nc.tensor.matmul(
                                stj,
                                k_t[pr][hp0:hp0 + 64, kt * 128:(kt + 1) * 128],
                                q_t[pr][hp0:hp0 + 64, lc * 512:(lc + 1) * 512],
                                start=True, stop=True,
                            )
                            pj = p2_cur[j][:, kt % 2, :]
                            on_scalar = ((j == 0 and kt not in D_BOTH)
                                         or (j == 1 and kt in S_BOTH))
                            if on_scalar:
                                nc.scalar.activation(
                                    out=pj, in_=stj, func=AF.Exp,
                                    scale=SCALE, bias=ebias_t)
                            else:
                                nc.vector.tensor_scalar(
                                    out=pj.bitcast(u8), in0=stj,
                                    scalar1=SCH_A8, scalar2=SCH_B8,
                                    op0=ALU.mult, op1=ALU.add,
                                )
                        for fn in pre_fillers.get(kt, ()):
                            fn()
                        # AV emission lags one kt-pair: hides the
                        # QK->exp->AV latency loop and the pair-boundary
                        # PSUM release
                        if kt % 2 == 1:
                            av_q.append((kt // 2, p2_cur))
                            if len(av_q) > 1:
                                emit_av(*av_q.pop(0))
                        for fn in fillers.get(kt, ()):
                            fn()
                        if kt in (5, 6) and pending is not None:
                            plc, ppr, puns, precs = pending
                            emit_divide_j(plc, ppr, puns, precs, kt - 5)
                            if kt == 6:
                                if ppr == NT - 1:
                                    proj_chunks = [
                                        (lambda l=plc, m=mo:
                                         emit_proj_chunk(l, m))
                                        for mo in range(NT)
                                    ]
                                pending = None
                        if kt in (8, 10, 12, 14) and proj_chunks:
                            proj_chunks.pop(0)()
                    for item in av_q:
                        emit_av(*item)
                    av_q = []
                    uns, recs = release_av(lc, pr, av)
                    pending = (lc, pr, uns, recs)
            plc, ppr, puns, precs = pending
            new_junk_target()
            for _ in range(4):
                emit_junk()
            emit_divide_j(plc, ppr, puns, precs, 0, fast=True)
            for _ in range(2):
                emit_junk()
            emit_divide_j(plc, ppr, puns, precs, 1, fast=True)
            for _ in range(2):
                emit_junk()
            for mo in range(NT):
                emit_proj_chunk(plc, mo)
                if mo < NT - 1:
                    emit_junk()

            if os.environ.get("ATTN_DBG"):
                dbg_a = nc.declare_dram_parameter(
                    "dbg_a", [C, LH], bf16, isOutput=True)
                dbg_k = nc.declare_dram_parameter(
                    "dbg_k", [C, L], bf16, isOutput=True)
                dbg_v = nc.declare_dram_parameter(
                    "dbg_v", [NKT // 2, 128, 2 * H * VE], fp8, isOutput=True)
                for t in range(NT):
                    nc.sync.dma_start(
                        out=dbg_a[t * 128:(t + 1) * 128, :], in_=attn_t[t])
                    nc.sync.dma_start(
                        out=dbg_k[t * 128:(t + 1) * 128, :], in_=k_t[t])
                for lt in range(NKT // 2):
                    nc.sync.dma_start(
                        out=dbg_v[lt],
                        in_=vT2[lt].rearrange("p o e -> p (o e)"))
    nc.compile()
    return nc


_NC = None


def _get_nc():
    global _NC
    if _NC is None:
        _NC = build_graph()
    return _NC


def _make_in_maps(x, norm_w, norm_b, qkv_w, qkv_b, proj_w, proj_b):
    bfl = ml_dtypes.bfloat16
    wqkvt = np.ascontiguousarray(qkv_w.T.astype(bfl))
    pwt = np.ascontiguousarray(proj_w.T.astype(bfl))
    vb = qkv_b[2 * C:3 * C].astype(np.float32)
    # v-bias folds into an effective proj bias (softmax rows sum to 1)
    pbe = proj_b.astype(np.float32) + proj_w.astype(np.float32) @ vb

    # packed per-channel vectors: vcs[r, t, v] = vec_v[t*128 + r]
    vcs = np.stack(
        [norm_w.astype(np.float32), norm_b.astype(np.float32),
         qkv_b[0:C].astype(np.float32), qkv_b[C:2 * C].astype(np.float32),
         pbe],
        axis=-1,
    ).reshape(NT, 128, 5).transpose(1, 0, 2)
    vcs = np.ascontiguousarray(vcs)

    indt = np.zeros((G, NT * 128), dtype=np.float32)
    for t in range(NT):
        indt[2 * t, t * 128:t * 128 + 64] = 1.0
        indt[2 * t + 1, t * 128 + 64:t * 128 + 128] = 1.0
    indt = indt.astype(bfl)

    shared = {"wqkvt": wqkvt, "pwt": pwt, "vcs": vcs, "indt": indt}
    in_maps = []
    for core in range(8):
        b, lh = core // 2, core % 2
        xb = np.asarray(x[b], dtype=np.float32)
        if lh:
            xb = np.concatenate([xb[:, LH:], xb[:, :LH]], axis=1)
        m = dict(shared)
        m["x"] = np.ascontiguousarray(xb.astype(bfl))
        m["xr"] = np.ascontiguousarray(xb[:, :LH])
        in_maps.append(m)
    return in_maps


def run(inputs, trace=False, tmpdir=None):
    from concourse.bass_utils import run_bass_kernel_spmd

    nc = _get_nc()
    in_maps = _make_in_maps(**inputs)
    res = run_bass_kernel_spmd(
        nc, in_maps, core_ids=list(range(8)), trace=trace, tmpdir=tmpdir
    )
    out = np.empty((B, C, L), dtype=np.float32)
    for core in range(8):
        b, lh = core // 2, core % 2
        out[b, :, lh * LH:(lh + 1) * LH] = res.results[core]["out"]
    return out, res


def kernel(**inputs):
    out, _ = run(inputs, trace=False)
    return out
